# revision 1
# baseline (speedup 1.0000x reference)
"""BiDAF attention layer on 8 Trainium2 NeuronCores (Bass/Tile).

Math (per batch b):
  t[i,j]  = sum_d (c[i,d]*w_cq[d] + w_q[d]) * q[j,d]   (= cq + sq0[j])
  a       = softmax_j(t)            (biases b_c/b_q/b_cq cancel in softmax)
  c2q     = a @ q
  m[i]    = max_j t[i,j];  sc0[i] = c[i,:]@w_c
  bvec    = softmax_i(m + sc0)      (biases cancel here too)
  q2c     = bvec @ c
  out     = [c | c2q | c*c2q | c*q2c]

Sharding: data-parallel over batch, 4 batches per core, params replicated.

Implementation notes:
  - w_q is folded into the transposed-c operand: chatT = w_cq*cT + w_q,
    applied for free in the PSUM evacuation (tensor_scalar mult+add with
    per-partition vectors). The q@w_q row term then emerges from the score
    contraction itself -- no separate sq0 computation, no exp bias.
  - Score/attention matmuls run in fp16 (fp32 matmul is 2 passes + 2
    LDWEIGHTS on TRN2; 16-bit is 1 pass + FWL), accumulating f32 PSUM.
    c stays f32 end-to-end for the output blocks and products.
  - cT/qT built via PE transposes (contraction over d needs d on
    partitions for both operands). The c PSUM is evacuated twice: affine
    -> chatT (scores) and plain -> cT (for the sc0 matvec).
  - Scores computed twice on PE: once as t [i,j] (row-max for bvec), once
    as tT [j,i] so ScalarE exp() lands e^T in SBUF in exactly the lhsT
    layout the c2q matmul needs (no e-transposes).
  - Softmax skips max-subtraction (|t| <= ~10, exp safe in f32/fp16) and
    the row sum l is fused into the c2q matmul as a ones column of rhs.
  - DMA is split across the three DGE paths to avoid head-of-line
    blocking: c-in/c-out on sync(SP), stage-out on scalar(ACT), q-in
    (with f32->fp16 cast) and c4-out on gpsimd(SWDGE).
"""

import sys

if "/opt/trn_rl_repo" not in sys.path:
    sys.path.insert(0, "/opt/trn_rl_repo")

import numpy as np

import concourse.bass as bass
import concourse.tile as tile
from concourse import bacc, mybir
from concourse.bass import ds, ts
from concourse.masks import make_identity

B, CL, QL, D = 32, 1024, 512, 256
NCORES = 8
BS = B // NCORES  # batches per core
P = 128
F32 = mybir.dt.float32
F16 = mybir.dt.float16

NT = CL // P  # 8 i-tiles
NJ = QL // P  # 4 j-chunks
ND = D // P   # 2 d-chunks
NH = 2        # i-halves for the [j,i]-layout score matmul
IH = CL // NH  # 512
KPH = NT // NH  # i-tiles per half

Exp = mybir.ActivationFunctionType.Exp
AxX = mybir.AxisListType.X
Mult = mybir.AluOpType.mult
Add = mybir.AluOpType.add


def build_bass(bs: int = BS):
    nc = bacc.Bacc(None)
    c_d = nc.declare_dram_parameter("c", [bs, CL, D], F32, isOutput=False)
    q_d = nc.declare_dram_parameter("q", [bs, QL, D], F32, isOutput=False)
    wc_d = nc.declare_dram_parameter("wc_cols", [P, ND], F16, isOutput=False)
    wq_d = nc.declare_dram_parameter("wq_cols", [P, ND], F32, isOutput=False)
    wcq_d = nc.declare_dram_parameter("wcq_cols", [P, ND], F32, isOutput=False)
    out_d = nc.declare_dram_parameter("out", [bs, CL, 4 * D], F32, isOutput=True)

    with tile.TileContext(nc) as tc:
        with (
            tc.tile_pool(name="consts", bufs=1) as consts,
            tc.tile_pool(name="io", bufs=3) as io,
            tc.tile_pool(name="ins", bufs=3) as ins,
            tc.tile_pool(name="work", bufs=3) as work,
            tc.tile_pool(name="ps_t", bufs=2, space="PSUM") as ps_t,
            tc.tile_pool(name="ps_tT", bufs=1, space="PSUM") as ps_tT,
            tc.tile_pool(name="ps_s", bufs=2, space="PSUM") as ps_s,
        ):
            ident_f = consts.tile([P, P], F32)
            ident_h = consts.tile([P, P], F16)
            ones_f = consts.tile([P, P], F32)
            ones_h = consts.tile([1, QL], F16)
            neg_shift = consts.tile([P, 1], F32)
            wc_sb = consts.tile([P, ND], F16)
            wq_sb = consts.tile([P, ND], F32)
            wcq_sb = consts.tile([P, ND], F32)

            def emit_inputs(b):
                # q loaded once, cast f32 -> fp16 in-flight (SWDGE)
                q_sb = ins.tile([P, NJ, D + 1], F16, tag="q_sb")
                nc.gpsimd.dma_start(
                    out=q_sb[:, :, 0:D],
                    in_=q_d[b].rearrange("(t p) d -> p t d", p=P),
                )
                nc.vector.memset(q_sb[:, :, D : D + 1], 1.0)
                c_sb = ins.tile([P, NT, D], F32, tag="c_sb")
                nc.sync.dma_start(
                    out=c_sb, in_=c_d[b].rearrange("(t p) d -> p t d", p=P)
                )
                # fp16 copy of c (re-read + cast) for transposes and q2c
                c_h = ins.tile([P, NT, D], F16, tag="c_h")
                nc.gpsimd.dma_start(
                    out=c_h, in_=c_d[b].rearrange("(t p) d -> p t d", p=P)
                )
                # output block 0 is just c; store straight from SBUF (SP ring)
                ov = out_d[b].rearrange("(t p) x -> p t x", p=P)
                nc.sync.dma_start(out=ov[:, :, 0:D], in_=c_sb)
                return c_sb, q_sb, c_h, ov

            pending = [emit_inputs(0)]

            for b in range(bs):
                c_sb, q_sb, c_h, ov = pending.pop(0)

                if b == 0:
                    nc.sync.dma_start(out=wc_sb, in_=wc_d[:])
                    nc.sync.dma_start(out=wq_sb, in_=wq_d[:])
                    nc.sync.dma_start(out=wcq_sb, in_=wcq_d[:])
                    make_identity(nc, ident_h)
                    make_identity(nc, ident_f)
                    nc.vector.memset(ones_f, 1.0)
                    nc.vector.memset(ones_h, 1.0)
                    nc.vector.memset(neg_shift, -2.5)
                else:
                    pass
                # prefetch up to two batches ahead of this batch's gpsimd
                # work so loads aren't FIFO-blocked behind c4 products
                if b == 0:
                    for nb in (1, 2):
                        if nb < bs:
                            pending.append(emit_inputs(nb))
                elif b + 2 < bs:
                    pending.append(emit_inputs(b + 2))

                # ------------- transpose q -> qT (fp16) -------------
                qT = work.tile([P, ND, QL], F16, tag="qT")
                for dc in range(ND):
                    pst = ps_t.tile([P, QL], F16, tag="t")
                    for jc in range(NJ):
                        nc.tensor.transpose(
                            pst[:, ts(jc, P)], q_sb[:, jc, ts(dc, P)], ident_h
                        )
                    if dc == 0:
                        nc.scalar.copy(qT[:, dc], pst)
                    else:
                        nc.vector.tensor_copy(qT[:, dc], pst)

                # ---- transpose c_h -> cT (plain) + chatT (affine) ----
                cT = work.tile([P, ND, CL], F16, tag="cT")
                chatT = work.tile([P, ND, CL], F16, tag="chatT")
                for dc in range(ND):
                    for h in range(NH):
                        pst = ps_t.tile([P, IH], F16, tag="t")
                        for k in range(KPH):
                            it = h * KPH + k
                            nc.tensor.transpose(
                                pst[:, ts(k, P)], c_h[:, it, ts(dc, P)], ident_h
                            )
                        sl = ds(h * IH, IH)
                        nc.vector.tensor_scalar(
                            out=chatT[:, dc, sl],
                            in0=pst,
                            scalar1=wcq_sb[:, dc : dc + 1],
                            scalar2=wq_sb[:, dc : dc + 1],
                            op0=Mult,
                            op1=Add,
                        )
                        nc.scalar.copy(cT[:, dc, sl], pst)

                # ---- sc0 rows: [1, IH] per half via M=1 matmuls (fp16) ----
                sc0_row = work.tile([1, CL], F16, tag="sc0r")
                for h in range(NH):
                    ps_sr = ps_t.tile([1, IH], F32, tag="t")
                    for dc in range(ND):
                        nc.tensor.matmul(
                            ps_sr,
                            wc_sb[:, dc : dc + 1],
                            cT[:, dc, ds(h * IH, IH)],
                            start=(dc == 0),
                            stop=(dc == ND - 1),
                        )
                    if h == 0:
                        nc.scalar.copy(sc0_row[0:1, ds(h * IH, IH)], ps_sr)
                    else:
                        nc.vector.tensor_copy(sc0_row[0:1, ds(h * IH, IH)], ps_sr)

                # ---- phase M: scores, e^T, and row maxes ----
                m_all = work.tile([P, NT], F32, tag="m_all")
                eTs = []
                for h in range(NH):
                    tTq = ps_tT.tile([P, NJ, IH], F32, tag="tTq")
                    for jc in range(NJ):
                        for dc in range(ND):
                            nc.tensor.matmul(
                                tTq[:, jc],
                                qT[:, dc, ts(jc, P)],
                                chatT[:, dc, ds(h * IH, IH)],
                                start=(dc == 0),
                                stop=(dc == ND - 1),
                            )
                    eT = work.tile([P, NJ, IH], F16, tag="eT")
                    eTs.append(eT)
                    for jc in range(NJ):
                        nc.scalar.activation(eT[:, jc], tTq[:, jc], Exp)

                    for k in range(KPH):
                        it = h * KPH + k
                        pt = ps_t.tile([P, QL], F32, tag="t")
                        for dc in range(ND):
                            nc.tensor.matmul(
                                pt,
                                chatT[:, dc, ts(it, P)],
                                qT[:, dc],
                                start=(dc == 0),
                                stop=False,
                            )
                        # + sc0[i] broadcast over j (K=1): m_all = max_j t + sc0
                        nc.tensor.matmul(
                            pt,
                            sc0_row[0:1, ts(it, P)],
                            ones_h,
                            start=False,
                            stop=True,
                        )
                        nc.vector.reduce_max(m_all[:, it : it + 1], pt, AxX)

                # ---- bvec numerators (ebv in fp16, shifted by -2.5) ----
                ebv_h = work.tile([P, NT], F16, tag="ebvh")
                nc.scalar.activation(ebv_h, m_all, Exp, bias=neg_shift[:, 0:1])
                colsum = work.tile([P, 1], F32, tag="colsum")
                nc.vector.reduce_sum(colsum, ebv_h, AxX)

                # ---- phase 2a: c2q matmuls for first half ----
                stage = io.tile([P, NT, 2 * D], F32, tag="stage")

                def mm2_tile(h, k):
                    it = h * KPH + k
                    po = ps_s.tile([P, D + 1], F32, tag="s")
                    for jc in range(NJ):
                        nc.tensor.matmul(
                            po,
                            eTs[h][:, jc, ts(k, P)],
                            q_sb[:, jc],
                            start=(jc == 0),
                            stop=(jc == NJ - 1),
                        )
                    linv = work.tile([P, 1], F32, tag="linv")
                    nc.vector.reciprocal(linv, po[:, D : D + 1])
                    nc.scalar.mul(stage[:, it, 0:D], po[:, 0:D], linv)
                    nc.vector.tensor_mul(
                        stage[:, it, D : 2 * D],
                        c_sb[:, it],
                        stage[:, it, 0:D],
                    )

                for k in range(KPH):
                    mm2_tile(0, k)
                nc.scalar.dma_start(
                    out=ov[:, 0:KPH, D : 3 * D], in_=stage[:, 0:KPH]
                )

                # ---- q2c chain (PE parts emitted after mm2 h0 so the
                # colsum/ebv dependencies are already satisfied) ----
                ps_tot = ps_s.tile([P, 1], F32, tag="s")
                nc.tensor.matmul(ps_tot, ones_f, colsum, start=True, stop=True)
                totinv = work.tile([P, 1], F32, tag="totinv")
                nc.vector.reciprocal(totinv, ps_tot)
                ps_q2c = ps_s.tile([1, D], F32, tag="s")
                for it in range(NT):
                    nc.tensor.matmul(
                        ps_q2c,
                        ebv_h[:, it : it + 1],
                        c_h[:, it],
                        start=(it == 0),
                        stop=(it == NT - 1),
                    )
                q2c_row = work.tile([1, D], F32, tag="q2cr")
                nc.vector.tensor_scalar_mul(q2c_row, ps_q2c, totinv[0:1, 0:1])
                ps_q2cb = ps_t.tile([P, D], F32, tag="t")
                nc.tensor.matmul(
                    ps_q2cb, ones_f[0:1, :], q2c_row, start=True, stop=True
                )
                q2c_sb = work.tile([P, D], F32, tag="q2csb")
                nc.scalar.copy(q2c_sb, ps_q2cb)
                c4st = io.tile([P, NT, D], F32, tag="c4st")
                for it in range(NT):
                    nc.gpsimd.tensor_mul(c4st[:, it], c_sb[:, it], q2c_sb)
                nc.gpsimd.dma_start(out=ov[:, :, 3 * D : 4 * D], in_=c4st)

                # ---- phase 2b: second half ----
                for k in range(KPH):
                    mm2_tile(1, k)
                nc.scalar.dma_start(
                    out=ov[:, KPH:NT, D : 3 * D], in_=stage[:, KPH:NT]
                )

    nc.compile()
    return nc


_NC_CACHE = {}


def _get_nc(bs: int = BS):
    if bs not in _NC_CACHE:
        _NC_CACHE[bs] = build_bass(bs)
    return _NC_CACHE[bs]


def _param_maps(w_c, w_q, w_cq):
    wc_cols = np.ascontiguousarray(
        np.asarray(w_c, np.float32).reshape(ND, P).T.astype(np.float16)
    )
    wq_cols = np.ascontiguousarray(np.asarray(w_q, np.float32).reshape(ND, P).T)
    wcq_cols = np.ascontiguousarray(
        np.asarray(w_cq, np.float32).reshape(ND, P).T
    )
    return wc_cols, wq_cols, wcq_cols


def _run(c, q, w_c, w_q, w_cq, trace=False, **trace_kwargs):
    from concourse.bass_utils import run_bass_kernel_spmd

    c = np.asarray(c, np.float32)
    q = np.asarray(q, np.float32)
    wc_cols, wq_cols, wcq_cols = _param_maps(w_c, w_q, w_cq)

    nc = _get_nc(BS)
    in_maps = []
    for k in range(NCORES):
        in_maps.append(
            {
                "c": np.ascontiguousarray(c[k * BS : (k + 1) * BS]),
                "q": np.ascontiguousarray(q[k * BS : (k + 1) * BS]),
                "wc_cols": wc_cols,
                "wq_cols": wq_cols,
                "wcq_cols": wcq_cols,
            }
        )
    res = None
    last_err = None
    for attempt in range(3):
        try:
            res = run_bass_kernel_spmd(
                nc,
                in_maps,
                core_ids=list(range(NCORES)),
                trace=trace,
                **trace_kwargs,
            )
            break
        except Exception as e:  # transient device wedges clear on retry
            last_err = e
            if "UNRECOVERABLE" not in str(e) and "UNAVAILABLE" not in str(e):
                raise
    if res is None:
        raise last_err
    out = np.concatenate([res.results[k]["out"] for k in range(NCORES)], axis=0)
    return out, res


def kernel(c, q, w_c, b_c, w_q, b_q, w_cq, b_cq):
    # b_c/b_q/b_cq provably cancel in both softmaxes; output doesn't use them.
    out, _ = _run(c, q, w_c, w_q, w_cq)
    return out



# revision 16
# speedup vs baseline: 1.1476x; 1.1476x over previous
"""BiDAF attention layer on 8 Trainium2 NeuronCores (Bass/Tile).

Math (per batch b):
  t[i,j]  = sum_d (c[i,d]*w_cq[d] + w_q[d]) * q[j,d]   (= cq + sq0[j])
  a       = softmax_j(t)            (biases b_c/b_q/b_cq cancel in softmax)
  c2q     = a @ q
  m[i]    = max_j t[i,j];  sc0[i] = c[i,:]@w_c
  bvec    = softmax_i(m + sc0)      (biases cancel here too)
  q2c     = bvec @ c
  out     = [c | c2q | c*c2q | c*q2c]

Sharding: data-parallel over batch, 4 batches per core, params replicated.

v2 design notes (vs the two-pass baseline):
  - Scores are computed ONCE, in [j,i] layout only.  The per-i row max is
    recovered from e = exp(t) itself: max_j t = log max_j e, and since
    bvec numerators are exp(m + sc0 - shift) = (max_j e) * exp(sc0-shift),
    no log is ever taken.  max_j e is a partition-axis max of eT, done as
    a free-dim max over the NJ chunks (DVE), then PE transposes of the
    [j,128] rows and a free-dim reduce_max.
  - c is loaded once (f32) straight into the output staging tile (block
    0 of the output row); fp16 copies for the PE are cast on-chip.  No
    second HBM read of c.
  - sc0 = c @ w_c is computed per-column with a fused DVE
    tensor_tensor_reduce against a broadcast w_c (no cT tensor at all).
  - Output rows are fully assembled in SBUF ([c | c2q | c*c2q | c*q2c])
    and stored with one 2 MiB DMA per half-batch -> 4 KiB contiguous
    descriptors instead of three interleaved strided stores.
  - chatT = w_cq*cT + w_q evacuation runs on the scalar engine
    (activation Identity with per-partition scale+bias vectors), keeping
    DVE free for the max/sc0/c2q element-wise work.
  - identity/ones/w_c-broadcast constants come in via one param DMA
    instead of gpsimd iota/affine_select at startup.
  - (po*linv)*c fused into one scalar_tensor_tensor op for block 2.
"""

import sys

if "/opt/trn_rl_repo" not in sys.path:
    sys.path.insert(0, "/opt/trn_rl_repo")

import numpy as np

import concourse.bass as bass
import concourse.tile as tile
from concourse import bacc, mybir
from concourse.bass import ds, ts

B, CL, QL, D = 32, 1024, 512, 256
NCORES = 8
BS = B // NCORES  # batches per core
P = 128
F32 = mybir.dt.float32
F16 = mybir.dt.float16

NT = CL // P  # 8 i-tiles
NJ = QL // P  # 4 j-chunks
ND = D // P   # 2 d-chunks
NH = 2        # i-halves
IH = CL // NH  # 512
KPH = NT // NH  # 4 i-tiles per half

Exp = mybir.ActivationFunctionType.Exp
Ident = mybir.ActivationFunctionType.Identity
AxX = mybir.AxisListType.X
Mult = mybir.AluOpType.mult
Add = mybir.AluOpType.add
Max = mybir.AluOpType.max

SHIFT = -2.5  # bvec numerator shift, keeps exp() in fp16-friendly range


def build_bass(bs: int = BS):
    nc = bacc.Bacc(None)
    c_d = nc.declare_dram_parameter("c", [bs, CL, D], F32, isOutput=False)
    q_d = nc.declare_dram_parameter("q", [bs, QL, D], F32, isOutput=False)
    ident_d = nc.declare_dram_parameter("ident_h", [P, P], F16, isOutput=False)
    ones_d = nc.declare_dram_parameter("ones_f", [P, P], F32, isOutput=False)
    wcb_d = nc.declare_dram_parameter("wc_bcast", [P, D], F16, isOutput=False)
    wq_d = nc.declare_dram_parameter("wq_cols", [P, ND], F32, isOutput=False)
    wcq_d = nc.declare_dram_parameter("wcq_cols", [P, ND], F32, isOutput=False)
    out_d = nc.declare_dram_parameter("out", [bs, CL, 4 * D], F32, isOutput=True)

    with tile.TileContext(nc) as tc:
        with (
            tc.tile_pool(name="consts", bufs=1) as consts,
            tc.tile_pool(name="stage", bufs=3) as stage_pool,
            tc.tile_pool(name="qin", bufs=3) as qin,
            tc.tile_pool(name="ch", bufs=2) as chp,
            tc.tile_pool(name="work", bufs=2) as work,
            tc.tile_pool(name="small", bufs=3) as small,
            tc.tile_pool(name="ps_sc", bufs=2, space="PSUM") as ps_sc,
            tc.tile_pool(name="ps_sm", bufs=2, space="PSUM") as ps_sm,
            tc.tile_pool(name="ps_po", bufs=2, space="PSUM") as ps_po,
        ):
            ident_h = consts.tile([P, P], F16)
            ones_f = consts.tile([P, P], F32)
            wcb_sb = consts.tile([P, D], F16)
            wq_sb = consts.tile([P, ND], F32)
            wcq_sb = consts.tile([P, ND], F32)
            neg_shift = consts.tile([P, 1], F32)

            def emit_inputs(b):
                # q loaded once, cast f32 -> fp16 in-flight (SWDGE)
                q_sb = qin.tile([P, NJ, D + 1], F16, tag="q_sb")
                nc.gpsimd.dma_start(
                    out=q_sb[:, :, 0:D],
                    in_=q_d[b].rearrange("(t p) d -> p t d", p=P),
                )
                nc.vector.memset(q_sb[:, :, D : D + 1], 1.0)
                # c lands directly in output block 0 of the staging rows
                stg = []
                for h in range(NH):
                    st = stage_pool.tile([P, KPH, 4 * D], F32, tag=f"stage{h}")
                    cv = c_d[b].rearrange("(t p) d -> p t d", p=P)
                    nc.sync.dma_start(
                        out=st[:, :, 0:D], in_=cv[:, ds(h * KPH, KPH)]
                    )
                    stg.append(st)
                ov = out_d[b].rearrange("(t p) x -> p t x", p=P)
                return q_sb, stg, ov

            def emit_prep(b, q_sb, stg):
                """casts + transposes + evacs + sc0 for batch b."""
                # fp16 copy of c for transposes / q2c rhs
                c_h = chp.tile([P, NT, D], F16, tag="c_h")
                for h in range(NH):
                    nc.scalar.copy(
                        c_h[:, ds(h * KPH, KPH)], stg[h][:, :, 0:D]
                    )
                # ---- transpose q -> qT (fp16) ----
                qT = work.tile([P, ND, QL], F16, tag="qT")
                for dc in range(ND):
                    pst = ps_sm.tile([P, QL], F16, tag="t")
                    for jc in range(NJ):
                        nc.tensor.transpose(
                            pst[:, ts(jc, P)], q_sb[:, jc, ts(dc, P)], ident_h
                        )
                    nc.vector.tensor_copy(qT[:, dc], pst)
                # ---- transpose c_h, evac as chatT = wcq*cT + wq (ACT) ----
                chatT = work.tile([P, ND, CL], F16, tag="chatT")
                for dc in range(ND):
                    for h in range(NH):
                        pst = ps_sm.tile([P, IH], F16, tag="t")
                        for k in range(KPH):
                            it = h * KPH + k
                            nc.tensor.transpose(
                                pst[:, ts(k, P)], c_h[:, it, ts(dc, P)], ident_h
                            )
                        nc.scalar.activation(
                            chatT[:, dc, ds(h * IH, IH)],
                            pst,
                            Ident,
                            bias=wq_sb[:, dc : dc + 1],
                            scale=wcq_sb[:, dc : dc + 1],
                        )
                # ---- sc0[i] = c[i,:] @ w_c, column layout (fused DVE) ----
                sc0_col = small.tile([P, NT], F32, tag="sc0")
                junk = small.tile([P, D], F16, tag="junk")
                for it in range(NT):
                    # fused multiply + per-partition sum via STT accum_out
                    nc.vector.scalar_tensor_tensor(
                        out=junk,
                        in0=c_h[:, it],
                        scalar=1.0,
                        in1=wcb_sb,
                        op0=Mult,
                        op1=Mult,
                        accum_out=sc0_col[:, it : it + 1],
                    )
                e_sc0 = small.tile([P, NT], F32, tag="esc0")
                nc.scalar.activation(e_sc0, sc0_col, Exp, bias=neg_shift[:, 0:1])
                return c_h, qT, chatT, e_sc0

            def emit_scores(b, prep):
                """tT scores + exp + NJ-chunk max; returns eT, emax halves."""
                c_h, qT, chatT, e_sc0 = prep
                eT = work.tile([P, NJ, CL], F16, tag="eT")
                emaxs = []
                for h in range(NH):
                    hsl = ds(h * IH, IH)
                    for jcp in range(NJ // 2):
                        pss = ps_sc.tile([P, 2, IH], F32, tag="s")
                        for j2 in range(2):
                            jc = jcp * 2 + j2
                            for dc in range(ND):
                                nc.tensor.matmul(
                                    pss[:, j2],
                                    qT[:, dc, ts(jc, P)],
                                    chatT[:, dc, hsl],
                                    start=(dc == 0),
                                    stop=(dc == ND - 1),
                                )
                        for j2 in range(2):
                            jc = jcp * 2 + j2
                            nc.scalar.activation(eT[:, jc, hsl], pss[:, j2], Exp)
                    # max over the NJ axis (free-dim TT max tree)
                    ma = small.tile([P, IH], F16, tag="ma")
                    mb = small.tile([P, IH], F16, tag="mb")
                    nc.vector.tensor_max(ma, eT[:, 0, hsl], eT[:, 1, hsl])
                    nc.vector.tensor_max(mb, eT[:, 2, hsl], eT[:, 3, hsl])
                    emax_h = small.tile([P, IH], F16, tag="emaxh")
                    nc.vector.tensor_max(emax_h, ma, mb)
                    emaxs.append(emax_h)
                return eT, emaxs

            def emit_bvec_q2c(b, prep, emaxs, stg):
                """partition-max via PE transposes, ebv, q2c, c4 products."""
                c_h, qT, chatT, e_sc0 = prep
                emax_col = small.tile([P, NT], F32, tag="emaxc")
                for h in range(NH):
                    pst = ps_sm.tile([P, KPH, P], F16, tag="t")
                    for k in range(KPH):
                        nc.tensor.transpose(
                            pst[:, k], emaxs[h][:, ts(k, P)], ident_h
                        )
                    nc.vector.reduce_max(
                        emax_col[:, ds(h * KPH, KPH)], pst, AxX
                    )
                # ebv = emax * exp(sc0+SHIFT);  colsum = sum_i(tile) ebv
                ebv_f = small.tile([P, NT], F32, tag="ebvf")
                colsum = small.tile([P, 1], F32, tag="colsum")
                nc.vector.scalar_tensor_tensor(
                    out=ebv_f,
                    in0=emax_col,
                    scalar=1.0,
                    in1=e_sc0,
                    op0=Mult,
                    op1=Mult,
                    accum_out=colsum,
                )
                ebv_h = small.tile([P, NT], F16, tag="ebvh")
                nc.vector.tensor_copy(ebv_h, ebv_f)
                # total = sum over partitions (PE), then 1/total
                ps_tot = ps_sm.tile([P, 1], F32, tag="t")
                nc.tensor.matmul(ps_tot, ones_f, colsum, start=True, stop=True)
                totinv = small.tile([P, 1], F32, tag="totinv")
                nc.vector.reciprocal(totinv, ps_tot)
                ps_q2c = ps_sm.tile([1, D], F32, tag="t")
                for it in range(NT):
                    nc.tensor.matmul(
                        ps_q2c,
                        ebv_h[:, it : it + 1],
                        c_h[:, it],
                        start=(it == 0),
                        stop=(it == NT - 1),
                    )
                q2c_row = small.tile([1, D], F32, tag="q2cr")
                nc.vector.tensor_scalar_mul(q2c_row, ps_q2c, totinv[0:1, 0:1])
                ps_q2cb = ps_sm.tile([P, D], F32, tag="t")
                nc.tensor.matmul(
                    ps_q2cb, ones_f[0:1, :], q2c_row, start=True, stop=True
                )
                q2c_sb = small.tile([P, D], F32, tag="q2csb")
                nc.scalar.copy(q2c_sb, ps_q2cb)
                return q2c_sb

            def emit_c2q_half(b, h, prep, eT, stg, q_sb, q2c_sb, ov, last):
                """c2q matmuls + all output blocks + store for half h."""
                c_h, qT, chatT, e_sc0 = prep
                st = stg[h]
                for k in range(KPH):
                    it = h * KPH + k
                    po = ps_po.tile([P, D + 1], F32, tag="po")
                    for jc in range(NJ):
                        nc.tensor.matmul(
                            po,
                            eT[:, jc, ts(it, P)],
                            q_sb[:, jc],
                            start=(jc == 0),
                            stop=(jc == NJ - 1),
                        )
                    linv = small.tile([P, 1], F32, tag="linv")
                    nc.vector.reciprocal(linv, po[:, D : D + 1])
                    nc.scalar.mul(st[:, k, D : 2 * D], po[:, 0:D], linv)
                    # block2 = (po * linv) * c   (one fused DVE op)
                    nc.vector.scalar_tensor_tensor(
                        out=st[:, k, 2 * D : 3 * D],
                        in0=po[:, 0:D],
                        scalar=linv,
                        in1=st[:, k, 0:D],
                        op0=Mult,
                        op1=Mult,
                    )
                    # block3 = c * q2c (gpsimd; vector on the final batch
                    # where everything else is idle)
                    eng = nc.vector if last else nc.gpsimd
                    eng.tensor_mul(
                        st[:, k, 3 * D : 4 * D], st[:, k, 0:D], q2c_sb
                    )
                nc.scalar.dma_start(
                    out=ov[:, ds(h * KPH, KPH)], in_=st
                )

            # ---------------- main schedule ----------------
            nc.vector.memset(neg_shift, SHIFT)
            nc.sync.dma_start(out=ident_h, in_=ident_d[:])
            nc.sync.dma_start(out=ones_f, in_=ones_d[:])
            nc.sync.dma_start(out=wcb_sb, in_=wcb_d[:])
            nc.sync.dma_start(out=wq_sb, in_=wq_d[:])
            nc.sync.dma_start(out=wcq_sb, in_=wcq_d[:])

            pending = [emit_inputs(0)]
            if bs > 1:
                pending.append(emit_inputs(1))
            preps = [emit_prep(0, pending[0][0], pending[0][1])]

            for b in range(bs):
                q_sb, stg, ov = pending.pop(0)
                prep = preps.pop(0)
                if b + 2 < bs:
                    pending.append(emit_inputs(b + 2))

                eT, emaxs = emit_scores(b, prep)

                # prep for b+1 lands here: PE transposes fill the gap while
                # ACT exps / DVE maxes of batch b run
                if b + 1 < bs:
                    nq, nstg, _ = pending[0]
                    preps.append(emit_prep(b + 1, nq, nstg))

                q2c_sb = emit_bvec_q2c(b, prep, emaxs, stg)
                last = b == bs - 1
                for h in range(NH):
                    emit_c2q_half(b, h, prep, eT, stg, q_sb, q2c_sb, ov, last)

    nc.compile()
    return nc


_NC_CACHE = {}


def _get_nc(bs: int = BS):
    if bs not in _NC_CACHE:
        _NC_CACHE[bs] = build_bass(bs)
    return _NC_CACHE[bs]


def _param_maps(w_c, w_q, w_cq):
    wc = np.asarray(w_c, np.float32)
    wc_bcast = np.ascontiguousarray(
        np.broadcast_to(wc.astype(np.float16), (P, D))
    )
    wq_cols = np.ascontiguousarray(np.asarray(w_q, np.float32).reshape(ND, P).T)
    wcq_cols = np.ascontiguousarray(
        np.asarray(w_cq, np.float32).reshape(ND, P).T
    )
    ident = np.eye(P, dtype=np.float16)
    ones = np.ones((P, P), dtype=np.float32)
    return wc_bcast, wq_cols, wcq_cols, ident, ones


def _run(c, q, w_c, w_q, w_cq, trace=False, **trace_kwargs):
    from concourse.bass_utils import run_bass_kernel_spmd

    c = np.asarray(c, np.float32)
    q = np.asarray(q, np.float32)
    wc_bcast, wq_cols, wcq_cols, ident, ones = _param_maps(w_c, w_q, w_cq)

    nc = _get_nc(BS)
    in_maps = []
    for k in range(NCORES):
        in_maps.append(
            {
                "c": np.ascontiguousarray(c[k * BS : (k + 1) * BS]),
                "q": np.ascontiguousarray(q[k * BS : (k + 1) * BS]),
                "ident_h": ident,
                "ones_f": ones,
                "wc_bcast": wc_bcast,
                "wq_cols": wq_cols,
                "wcq_cols": wcq_cols,
            }
        )
    res = None
    last_err = None
    for attempt in range(3):
        try:
            res = run_bass_kernel_spmd(
                nc,
                in_maps,
                core_ids=list(range(NCORES)),
                trace=trace,
                **trace_kwargs,
            )
            break
        except Exception as e:  # transient device wedges clear on retry
            last_err = e
            if "UNRECOVERABLE" not in str(e) and "UNAVAILABLE" not in str(e):
                raise
    if res is None:
        raise last_err
    out = np.concatenate([res.results[k]["out"] for k in range(NCORES)], axis=0)
    return out, res


def kernel(c, q, w_c, b_c, w_q, b_q, w_cq, b_cq):
    # b_c/b_q/b_cq provably cancel in both softmaxes; output doesn't use them.
    out, _ = _run(c, q, w_c, w_q, w_cq)
    return out


# revision 21
# speedup vs baseline: 1.2745x; 1.1106x over previous
"""BiDAF attention layer on 8 Trainium2 NeuronCores (Bass/Tile).

Math (per batch b):
  t[i,j]  = sum_d (c[i,d]*w_cq[d] + w_q[d]) * q[j,d]   (= cq + sq0[j])
  a       = softmax_j(t)            (biases b_c/b_q/b_cq cancel in softmax)
  c2q     = a @ q
  m[i]    = max_j t[i,j];  sc0[i] = c[i,:]@w_c
  bvec    = softmax_i(m + sc0)      (biases cancel here too)
  q2c     = bvec @ c
  out     = [c | c2q | c*c2q | c*q2c]

Sharding: data-parallel over batch, 4 batches per core, params replicated.

Design notes:
  - Scores are computed ONCE, in [j,i] layout only.  The per-i row max is
    recovered from e = exp(t) itself: since bvec numerators are
    exp(m + sc0 - shift) = (max_j e) * exp(sc0 - shift), no log is needed.
    max_j e is a partition-axis max of eT: free-dim max over the NJ chunks
    (DVE) -> PE transposes of the [j,128] rows -> free-dim reduce_max.
  - c is loaded once (f32) straight into block 0 of the output staging
    rows; the fp16 copy for the PE is cast on-chip (no second HBM read).
  - sc0 = c @ w_c per-column via fused DVE scalar_tensor_tensor accum_out.
  - Blocks 0..2 of each output row are staged in SBUF and stored with one
    3 KiB-descriptor DMA per half-batch as soon as c2q for that half is
    done (no q2c dependency); block 3 (c*q2c) is stored separately per
    half on the sync ring.
  - PE order keeps the array warm (HAM) through the bvec latency chain:
    scores(b) -> prep transposes(b+1) -> c2q h0(b) -> emax transposes(b)
    -> q2c(b) -> c2q h1(b) -> scores(b+1).
  - chatT = w_cq*cT + w_q evacuation runs on ACT (activation Identity
    with per-partition scale+bias); block1 = po/l on ACT; block2 =
    (po/l)*c fused on DVE; c4 = c*q2c on GpSimd (DVE for the last batch).
  - Batch 0's q comes over the sync ring as f32 + DVE cast (SWDGE
    descriptor generation would delay it to ~10us); later batches use
    SWDGE cast-in-flight loads.  Params ride the scalar ring, packed into
    two DMAs, so c(b0) heads the sync ring.
"""

import sys

if "/opt/trn_rl_repo" not in sys.path:
    sys.path.insert(0, "/opt/trn_rl_repo")

import numpy as np

import concourse.bass as bass
import concourse.tile as tile
from concourse import bacc, mybir
from concourse.bass import ds, ts

B, CL, QL, D = 32, 1024, 512, 256
NCORES = 8
BS = B // NCORES  # batches per core
P = 128
F32 = mybir.dt.float32
F16 = mybir.dt.float16

NT = CL // P  # 8 i-tiles
NJ = QL // P  # 4 j-chunks
ND = D // P   # 2 d-chunks
NH = 2        # i-halves
IH = CL // NH  # 512
KPH = NT // NH  # 4 i-tiles per half

Exp = mybir.ActivationFunctionType.Exp
Ident = mybir.ActivationFunctionType.Identity
AxX = mybir.AxisListType.X
Mult = mybir.AluOpType.mult
Add = mybir.AluOpType.add

SHIFT = -2.5  # bvec numerator shift, keeps exp() in fp16-friendly range


def build_bass(bs: int = BS):
    nc = bacc.Bacc(None)
    c_d = nc.declare_dram_parameter("c", [bs, CL, D], F32, isOutput=False)
    q_d = nc.declare_dram_parameter("q", [bs, QL, D], F32, isOutput=False)
    # params packed host-side: fp16 [ident | wc_bcast], f32 [ones | wq | wcq]
    ph_d = nc.declare_dram_parameter("params_h", [P, P + D], F16, isOutput=False)
    pf_d = nc.declare_dram_parameter(
        "params_f", [P, P + 2 * ND + 1], F32, isOutput=False
    )
    out_d = nc.declare_dram_parameter("out", [bs, CL, 4 * D], F32, isOutput=True)

    with tile.TileContext(nc) as tc:
        with (
            tc.tile_pool(name="consts", bufs=1) as consts,
            tc.tile_pool(name="stage", bufs=3) as stage_pool,
            tc.tile_pool(name="c4p", bufs=2) as c4p,
            tc.tile_pool(name="qin", bufs=3) as qin,
            tc.tile_pool(name="ch", bufs=2) as chp,
            tc.tile_pool(name="work", bufs=2) as work,
            tc.tile_pool(name="small", bufs=3) as small,
            tc.tile_pool(name="ps_sc", bufs=3, space="PSUM") as ps_sc,
            tc.tile_pool(name="ps_sm", bufs=2, space="PSUM") as ps_sm,
            tc.tile_pool(name="ps_po", bufs=3, space="PSUM") as ps_po,
        ):
            par_h = consts.tile([P, P + D], F16)
            par_f = consts.tile([P, P + 2 * ND + 1], F32)
            ident_h = par_h[:, 0:P]
            wcb_sb = par_h[:, P : P + D]
            ones_f = par_f[:, 0:P]
            wq_sb = par_f[:, P : P + ND]
            wcq_sb = par_f[:, P + ND : P + 2 * ND]
            neg_shift = par_f[:, P + 2 * ND : P + 2 * ND + 1]

            def emit_inputs(b):
                stg = []
                for h in range(NH):
                    st = stage_pool.tile([P, KPH, 3 * D], F32, tag=f"stage{h}")
                    cv = c_d[b].rearrange("(t p) d -> p t d", p=P)
                    nc.sync.dma_start(
                        out=st[:, :, 0:D], in_=cv[:, ds(h * KPH, KPH)]
                    )
                    stg.append(st)
                q_sb = qin.tile([P, NJ, D + 1], F16, tag="q_sb")
                if b == 0:
                    # sync-ring f32 load + DVE cast: ready ~6us before the
                    # SWDGE path can deliver it
                    q_f = qin.tile([P, NJ, D], F32, tag="q_f")
                    nc.sync.dma_start(
                        out=q_f, in_=q_d[b].rearrange("(t p) d -> p t d", p=P)
                    )
                    nc.vector.tensor_copy(q_sb[:, :, 0:D], q_f)
                else:
                    nc.gpsimd.dma_start(
                        out=q_sb[:, :, 0:D],
                        in_=q_d[b].rearrange("(t p) d -> p t d", p=P),
                    )
                nc.vector.memset(q_sb[:, :, D : D + 1], 1.0)
                ov = out_d[b].rearrange("(t p) x -> p t x", p=P)
                return q_sb, stg, ov

            def emit_prep(b, q_sb, stg):
                """casts + transposes + evacs for batch b (PE-heavy part)."""
                c_h = chp.tile([P, NT, D], F16, tag="c_h")
                for h in range(NH):
                    nc.scalar.copy(
                        c_h[:, ds(h * KPH, KPH)], stg[h][:, :, 0:D]
                    )
                qT = work.tile([P, ND, QL], F16, tag="qT")
                for dc in range(ND):
                    pst = ps_sm.tile([P, QL], F16, tag="t")
                    for jc in range(NJ):
                        nc.tensor.transpose(
                            pst[:, ts(jc, P)], q_sb[:, jc, ts(dc, P)], ident_h
                        )
                    nc.vector.tensor_copy(qT[:, dc], pst)
                chatT = work.tile([P, ND, CL], F16, tag="chatT")
                for dc in range(ND):
                    for h in range(NH):
                        pst = ps_sm.tile([P, IH], F16, tag="t")
                        for k in range(KPH):
                            it = h * KPH + k
                            nc.tensor.transpose(
                                pst[:, ts(k, P)], c_h[:, it, ts(dc, P)], ident_h
                            )
                        nc.scalar.activation(
                            chatT[:, dc, ds(h * IH, IH)],
                            pst,
                            Ident,
                            bias=wq_sb[:, dc : dc + 1],
                            scale=wcq_sb[:, dc : dc + 1],
                        )
                return c_h, qT, chatT

            def emit_sc0(b, prep):
                """sc0 = c @ w_c (fused DVE) + exp; late, off the hot path."""
                c_h, qT, chatT = prep
                sc0_col = small.tile([P, NT], F32, tag="sc0")
                junk = small.tile([P, D], F16, tag="junk")
                for it in range(NT):
                    nc.vector.scalar_tensor_tensor(
                        out=junk,
                        in0=c_h[:, it],
                        scalar=1.0,
                        in1=wcb_sb,
                        op0=Mult,
                        op1=Mult,
                        accum_out=sc0_col[:, it : it + 1],
                    )
                e_sc0 = small.tile([P, NT], F32, tag="esc0")
                nc.scalar.activation(e_sc0, sc0_col, Exp, bias=neg_shift)
                return e_sc0

            def emit_scores(b, prep):
                """tT scores + exp + NJ-chunk max; returns eT, emax halves."""
                c_h, qT, chatT = prep
                eT = work.tile([P, NJ, CL], F16, tag="eT")
                emaxs = []
                for h in range(NH):
                    hsl = ds(h * IH, IH)
                    for jc in range(NJ):
                        pss = ps_sc.tile([P, IH], F32, tag="s")
                        for dc in range(ND):
                            nc.tensor.matmul(
                                pss,
                                qT[:, dc, ts(jc, P)],
                                chatT[:, dc, hsl],
                                start=(dc == 0),
                                stop=(dc == ND - 1),
                            )
                        nc.scalar.activation(eT[:, jc, hsl], pss, Exp)
                    ma = small.tile([P, IH], F16, tag="ma")
                    mb = small.tile([P, IH], F16, tag="mb")
                    nc.vector.tensor_max(ma, eT[:, 0, hsl], eT[:, 1, hsl])
                    nc.vector.tensor_max(mb, eT[:, 2, hsl], eT[:, 3, hsl])
                    emax_h = small.tile([P, IH], F16, tag="emaxh")
                    nc.vector.tensor_max(emax_h, ma, mb)
                    emaxs.append(emax_h)
                return eT, emaxs

            def emit_bvec(b, e_sc0, emaxs):
                """partition-max via PE transposes + ebv/colsum (DVE)."""
                emax_col = small.tile([P, NT], F32, tag="emaxc")
                for h in range(NH):
                    pst = ps_sm.tile([P, KPH, P], F16, tag="t")
                    for k in range(KPH):
                        nc.tensor.transpose(
                            pst[:, k], emaxs[h][:, ts(k, P)], ident_h
                        )
                    nc.vector.reduce_max(
                        emax_col[:, ds(h * KPH, KPH)], pst, AxX
                    )
                ebv_f = small.tile([P, NT], F32, tag="ebvf")
                colsum = small.tile([P, 1], F32, tag="colsum")
                nc.vector.scalar_tensor_tensor(
                    out=ebv_f,
                    in0=emax_col,
                    scalar=1.0,
                    in1=e_sc0,
                    op0=Mult,
                    op1=Mult,
                    accum_out=colsum,
                )
                ebv_h = small.tile([P, NT], F16, tag="ebvh")
                nc.vector.tensor_copy(ebv_h, ebv_f)
                return ebv_h, colsum

            def emit_q2c(b, prep, ebv_h, colsum):
                """q2c = (ebv @ c) / total, broadcast to all partitions."""
                c_h, qT, chatT = prep
                ps_tot = ps_sm.tile([P, 1], F32, tag="t")
                nc.tensor.matmul(ps_tot, ones_f, colsum, start=True, stop=True)
                totinv = small.tile([P, 1], F32, tag="totinv")
                nc.vector.reciprocal(totinv, ps_tot)
                ps_q2c = ps_sm.tile([1, D], F32, tag="t")
                for it in range(NT):
                    nc.tensor.matmul(
                        ps_q2c,
                        ebv_h[:, it : it + 1],
                        c_h[:, it],
                        start=(it == 0),
                        stop=(it == NT - 1),
                    )
                q2c_row = small.tile([1, D], F32, tag="q2cr")
                nc.vector.tensor_scalar_mul(q2c_row, ps_q2c, totinv[0:1, 0:1])
                ps_q2cb = ps_sm.tile([P, D], F32, tag="t")
                nc.tensor.matmul(
                    ps_q2cb, ones_f[0:1, :], q2c_row, start=True, stop=True
                )
                q2c_sb = small.tile([P, D], F32, tag="q2csb")
                nc.scalar.copy(q2c_sb, ps_q2cb)
                return q2c_sb

            def emit_c2q(b, h, eT, q_sb, stg, ov):
                """c2q matmuls + blocks 1+2 evac + store blocks 0..2.

                MM groups and evacs interleave so at most 3 po tiles are
                live (pool bufs=3) and the PE never waits on an evac that
                has not been emitted yet."""
                st = stg[h]
                pos = []

                def mm(k):
                    it = h * KPH + k
                    po = ps_po.tile([P, D + 1], F32, tag="po")
                    for jc in range(NJ):
                        nc.tensor.matmul(
                            po,
                            eT[:, jc, ts(it, P)],
                            q_sb[:, jc],
                            start=(jc == 0),
                            stop=(jc == NJ - 1),
                        )
                    pos.append(po)

                def evac(k):
                    po = pos[k]
                    linv = small.tile([P, 1], F32, tag="linv")
                    nc.vector.reciprocal(linv, po[:, D : D + 1])
                    nc.scalar.mul(st[:, k, D : 2 * D], po[:, 0:D], linv)
                    nc.vector.scalar_tensor_tensor(
                        out=st[:, k, 2 * D : 3 * D],
                        in0=po[:, 0:D],
                        scalar=linv,
                        in1=st[:, k, 0:D],
                        op0=Mult,
                        op1=Mult,
                    )

                mm(0)
                mm(1)
                evac(0)
                mm(2)
                evac(1)
                mm(3)
                evac(2)
                evac(3)
                nc.scalar.dma_start(
                    out=ov[:, ds(h * KPH, KPH), 0 : 3 * D], in_=st
                )

            def emit_c4(b, h, stg, q2c_sb, ov, last):
                """block3 = c * q2c for half h + store on the sync ring."""
                c4t = c4p.tile([P, KPH, D], F32, tag=f"c4_{h}")
                eng = nc.vector if last else nc.gpsimd
                for k in range(KPH):
                    eng.tensor_mul(c4t[:, k], stg[h][:, k, 0:D], q2c_sb)
                nc.sync.dma_start(
                    out=ov[:, ds(h * KPH, KPH), 3 * D : 4 * D], in_=c4t
                )

            # ---------------- main schedule ----------------
            nc.scalar.dma_start(out=par_h, in_=ph_d[:])
            nc.scalar.dma_start(out=par_f, in_=pf_d[:])

            pending = [emit_inputs(0)]
            if bs > 1:
                pending.append(emit_inputs(1))
            preps = [emit_prep(0, pending[0][0], pending[0][1])]
            esc0s = [emit_sc0(0, preps[0])]

            for b in range(bs):
                q_sb, stg, ov = pending.pop(0)
                prep = preps.pop(0)
                e_sc0 = esc0s.pop(0)
                last = b == bs - 1
                if b + 2 < bs:
                    pending.append(emit_inputs(b + 2))

                eT, emaxs = emit_scores(b, prep)

                # PE keeps running: next batch's transposes cover the
                # exp/NJ-max latency, then emax transposes, then c2q
                if not last:
                    nq, nstg, _ = pending[0]
                    preps.append(emit_prep(b + 1, nq, nstg))

                ebv_h, colsum = emit_bvec(b, e_sc0, emaxs)
                emit_c2q(b, 0, eT, q_sb, stg, ov)
                q2c_sb = emit_q2c(b, prep, ebv_h, colsum)
                emit_c4(b, 0, stg, q2c_sb, ov, last)

                if not last:
                    esc0s.append(emit_sc0(b + 1, preps[0]))

                emit_c2q(b, 1, eT, q_sb, stg, ov)
                emit_c4(b, 1, stg, q2c_sb, ov, last)

    nc.compile()
    return nc


_NC_CACHE = {}


def _get_nc(bs: int = BS):
    if bs not in _NC_CACHE:
        _NC_CACHE[bs] = build_bass(bs)
    return _NC_CACHE[bs]


def _param_maps(w_c, w_q, w_cq):
    wc = np.asarray(w_c, np.float32)
    params_h = np.concatenate(
        [np.eye(P, dtype=np.float16),
         np.broadcast_to(wc.astype(np.float16), (P, D))],
        axis=1,
    )
    wq_cols = np.asarray(w_q, np.float32).reshape(ND, P).T
    wcq_cols = np.asarray(w_cq, np.float32).reshape(ND, P).T
    params_f = np.concatenate(
        [np.ones((P, P), np.float32), wq_cols, wcq_cols,
         np.full((P, 1), SHIFT, np.float32)],
        axis=1,
    )
    return np.ascontiguousarray(params_h), np.ascontiguousarray(params_f)


def _run(c, q, w_c, w_q, w_cq, trace=False, **trace_kwargs):
    from concourse.bass_utils import run_bass_kernel_spmd

    c = np.asarray(c, np.float32)
    q = np.asarray(q, np.float32)
    params_h, params_f = _param_maps(w_c, w_q, w_cq)

    nc = _get_nc(BS)
    in_maps = []
    for k in range(NCORES):
        in_maps.append(
            {
                "c": np.ascontiguousarray(c[k * BS : (k + 1) * BS]),
                "q": np.ascontiguousarray(q[k * BS : (k + 1) * BS]),
                "params_h": params_h,
                "params_f": params_f,
            }
        )
    res = None
    last_err = None
    for attempt in range(3):
        try:
            res = run_bass_kernel_spmd(
                nc,
                in_maps,
                core_ids=list(range(NCORES)),
                trace=trace,
                **trace_kwargs,
            )
            break
        except Exception as e:  # transient device wedges clear on retry
            last_err = e
            if "UNRECOVERABLE" not in str(e) and "UNAVAILABLE" not in str(e):
                raise
    if res is None:
        raise last_err
    out = np.concatenate([res.results[k]["out"] for k in range(NCORES)], axis=0)
    return out, res


def kernel(c, q, w_c, b_c, w_q, b_q, w_cq, b_cq):
    # b_c/b_q/b_cq provably cancel in both softmaxes; output doesn't use them.
    out, _ = _run(c, q, w_c, w_q, w_cq)
    return out
